# revision 5
# baseline (speedup 1.0000x reference)
"""DPPConv2d Trainium2 Bass kernel (v3).

Reference computation (per sample s):
  pooled = mean_{h,w} x[s]                              [Cin]
  h      = relu(pooled @ W1.T)                          [hidden]
  logits = h @ W2.T + b2                                [P*Cout]
  attn   = softmax(logits.reshape(P, Cout) / 0.5, p)    [P, Cout]
  m      = (mean_{o,i}(|W[p,:,:,k,l]| - thr[p,:]) > 0)  [P, K, K]
  agg    = sum_p attn[p, co] * m[p, kl] * W[p, co, ci, kl]
  out[s] = conv2d(x[s], agg, pad=1)                     [Cout, H, W]

Sharding: data-parallel over batch -- 8 cores x 4 samples each.

Host prep (weight-only / layout-only, no x-dependent compute):
  - x zero-padded to 66x66 (f32; conv runs f32r self-loading matmuls).
  - binary mask m computed from (weight, threshold) alone and folded
    into the bank; bank shipped in [co, p, ci, kl] layout.
  - psa_w1 shipped pre-transposed; psa_w2.T shipped directly; bias as
    a separate 1-row tensor added via an accumulating 1-row matmul.

Device pipeline per core (x-dependent compute only):
  - pooled sums via split DVE reduces; SE-MLP via PE matmuls; softmax
    over P without max-subtraction (logits are O(1) at this scale).
  - attn -> [co, p, s] per-partition scalars via 4 small PE transposes.
  - phase A (all samples, hoisted ahead of all convs): agg_s via bf16
    scalar_tensor_tensor FMA chains in [co, p, ci*kl] layout, split in
    kl-halves so transposes start early; 9 bf16 PE transposes/sample
    -> conv lhsT [ci, kl, co] (f32r).
  - phase B: conv as 9 shifted accumulating f32r matmuls per 8-row
    chunk (self-loading -- no LDWEIGHTS); PSUM->SBUF on ScalarE; DMA
    out in bf16 (host casts back to f32).

Scheduling: each rep's front chain is shifted one rep earlier in Tile
scheduler priority (tc.high_priority) so it fills the previous rep's
convolution block; the steady-state schedule is PE-dense (zero PE gaps
in the scheduling-sim trace) at ~65 us/pass vs the ~61.4 us f32r conv
roofline.
"""

import os
import sys

try:
    import concourse.bass as bass  # noqa: F401
except Exception:  # pragma: no cover
    sys.path.insert(0, "/opt/trn_rl_repo")

from contextlib import ExitStack

import numpy as np

import concourse.bass as bass
import concourse.tile as tile
from concourse import mybir
from concourse.bass_utils import run_bass_kernel_spmd

N_CORES = 8
BS = 32
BS_LOCAL = BS // N_CORES  # 4
CIN = 128
COUT = 128
H = W = 64
P_PAT = 4
KS = 3
KK = KS * KS
HID = 33
TEMP = 0.5
YC = 8          # output rows per conv chunk
N_CHUNK = H // YC
PRIO_SHIFT = int(os.environ.get("DPP_PRIO_SHIFT", "800"))

F32 = mybir.dt.float32
F32R = mybir.dt.float32r
BF16 = mybir.dt.bfloat16


def build_nc(use_f32r=True, rep=1, mm=None, knobs=()):
    nc = bass.Bass("TRN2", target_bir_lowering=False, debug=False,
                   num_swdge_queues=4)

    XF32 = os.environ.get("DPP_XF32", "0") == "1"
    x_d = nc.dram_tensor("x", [BS_LOCAL, CIN, H + 2, W + 2],
                         F32 if XF32 else BF16, kind="ExternalInput")
    wm_d = nc.dram_tensor("wm", [COUT, P_PAT, CIN, KK], BF16,
                          kind="ExternalInput")
    w1t_d = nc.dram_tensor("w1t", [CIN, HID], F32, kind="ExternalInput")
    w2b_d = nc.dram_tensor("w2b", [HID, P_PAT * COUT], F32,
                           kind="ExternalInput")
    b2_d = nc.dram_tensor("b2", [1, P_PAT * COUT], F32, kind="ExternalInput")
    id_d = nc.dram_tensor("ident", [128, 128], F32, kind="ExternalInput")
    out_d = nc.dram_tensor("out", [BS_LOCAL, COUT, H, W], BF16,
                           kind="ExternalOutput")
    K = set(knobs)

    with tile.TileContext(nc) as tc, ExitStack() as ctx:
        consts = ctx.enter_context(tc.tile_pool(name="consts", bufs=1))
        xpool = ctx.enter_context(tc.tile_pool(name="xpool", bufs=2))
        wpool = ctx.enter_context(tc.tile_pool(name="wpool", bufs=1))
        mlpp = ctx.enter_context(tc.tile_pool(name="mlpp", bufs=1))
        aggp = ctx.enter_context(tc.tile_pool(name="aggp", bufs=2))
        lhsp = ctx.enter_context(tc.tile_pool(name="lhsp", bufs=4))
        outp = ctx.enter_context(tc.tile_pool(name="outp", bufs=4))
        ps_small = ctx.enter_context(
            tc.tile_pool(name="ps_small", bufs=2, space="PSUM"))
        ps_tp = ctx.enter_context(
            tc.tile_pool(name="ps_tp", bufs=2, space="PSUM"))
        ps_mm = ctx.enter_context(
            tc.tile_pool(name="ps_mm", bufs=4, space="PSUM"))

        def pe_absorb(ap_col):
            """Tiny matmul whose only new dependency is ap_col's producer.

            The fp32/fp32r self-loading PE matmul encoding has a single
            sync-wait slot, so each PE instruction may introduce at most
            one new vector-clock dependency. These 1-column matmuls make
            the PE observe one clock (e.g. a DMA queue) ahead of the real
            matmul that would otherwise need two waits.
            """
            d_ps = ps_small.tile([1, 1], F32, tag="sm", name="dummy_ps")
            nc.tensor.matmul(d_ps[:], ap_col, ap_col)

        # ---- persistent constants ---------------------------------------
        ident = consts.tile([128, 128], F32, tag="ident")
        nc.sync.dma_start(ident[:], id_d[:])
        identb = consts.tile([128, 128], BF16, tag="identb")
        nc.scalar.copy(identb[:], ident[:])
        ones1f = consts.tile([1, BS_LOCAL], F32, tag="ones1f")
        nc.vector.memset(ones1f[:], 1.0)
        ones1 = consts.tile([1, BS_LOCAL], F32R, tag="ones1")
        nc.scalar.copy(ones1[:], ones1f[:])

        from contextlib import nullcontext
        for _rep in range(rep):
            # Hoist each rep's front chain (DMAs, pooled, MLP, softmax,
            # attn transposes, aggregation, weight transposes) one rep
            # earlier in scheduler priority so it overlaps the previous
            # rep's convolution block instead of queueing behind it.
            hoist = (tc.high_priority(offset=PRIO_SHIFT) if _rep > 0
                     else nullcontext())
            ctx_front = ExitStack()
            ctx_front.enter_context(hoist)
            # ---- input DMAs ---------------------------------------------
            xs = [xpool.tile([CIN, H + 2, W + 2], F32R, tag=f"xs{s}",
                             name=f"xs{s}") for s in range(BS_LOCAL)]
            for s in range(BS_LOCAL):
                if XF32:
                    nc.scalar.dma_start(xs[s][:].bitcast(F32), x_d[s])
                else:
                    nc.gpsimd.dma_start(xs[s][:], x_d[s])
            wm_sb = wpool.tile([COUT, P_PAT, CIN * KK], BF16, tag="wm")
            nc.sync.dma_start(
                wm_sb[:], wm_d[:].rearrange("co p ci kl -> co p (ci kl)"))
            w1t_sb = mlpp.tile([CIN, HID], F32, tag="w1t")
            nc.sync.dma_start(w1t_sb[:], w1t_d[:])
            w2b_sb = mlpp.tile([HID, P_PAT * COUT], F32R, tag="w2b")
            nc.gpsimd.dma_start(w2b_sb[:], w2b_d[:])
            b2_sb = mlpp.tile([1, P_PAT * COUT], F32R, tag="b2")
            nc.gpsimd.dma_start(b2_sb[:], b2_d[:])

            # ---- SE attention MLP (batched over the 4 local samples) ----
            pooled = mlpp.tile([CIN, BS_LOCAL], F32, tag="pooled")
            pooled_h = mlpp.tile([CIN, BS_LOCAL], F32, tag="pooled_h")
            for s in range(BS_LOCAL):
                nc.vector.reduce_sum(
                    pooled_h[:, s:s + 1], xs[s][:, 0:33, :].bitcast(F32),
                    axis=mybir.AxisListType.XY)
                nc.vector.reduce_sum(
                    pooled[:, s:s + 1], xs[s][:, 33:66, :].bitcast(F32),
                    axis=mybir.AxisListType.XY)
            nc.vector.tensor_add(pooled[:], pooled[:], pooled_h[:])

            pe_absorb(pooled[:, 0:1])
            h_ps = ps_small.tile([HID, BS_LOCAL], F32, tag="sm")
            nc.tensor.matmul(h_ps[:], w1t_sb[:], pooled[:])
            h_sb = mlpp.tile([HID, BS_LOCAL], F32R, tag="h_sb")
            nc.scalar.activation(
                h_sb[:], h_ps[:], mybir.ActivationFunctionType.Relu,
                scale=1.0 / (H * W))

            lg_ps = ps_small.tile([BS_LOCAL, P_PAT, COUT], F32, tag="sm")
            nc.tensor.matmul(
                lg_ps[:].rearrange("s p c -> s (p c)"),
                h_sb[:], w2b_sb[:], start=True, stop=False)
            nc.tensor.matmul(
                lg_ps[:].rearrange("s p c -> s (p c)"),
                ones1[:], b2_sb[:], start=False, stop=True)

            # softmax over the pattern axis (temperature 0.5 -> scale 2.0;
            # logits are O(1) here so the max-subtraction is skipped)
            sm_e = mlpp.tile([BS_LOCAL, P_PAT, COUT], F32, tag="sm_e")
            for p in range(P_PAT):
                nc.scalar.activation(
                    sm_e[:, p], lg_ps[:, p], mybir.ActivationFunctionType.Exp,
                    scale=1.0 / TEMP)
            sm_sum = mlpp.tile([BS_LOCAL, COUT], F32, tag="sm_sum")
            nc.vector.tensor_add(sm_sum[:], sm_e[:, 0], sm_e[:, 1])
            nc.vector.tensor_add(sm_sum[:], sm_sum[:], sm_e[:, 2])
            nc.vector.tensor_add(sm_sum[:], sm_sum[:], sm_e[:, 3])
            sm_rec = mlpp.tile([BS_LOCAL, COUT], F32, tag="sm_rec")
            nc.vector.reciprocal(sm_rec[:], sm_sum[:])
            attn_sb = mlpp.tile([BS_LOCAL, P_PAT, COUT], F32, tag="attn_sb")
            for p in range(P_PAT):
                nc.vector.tensor_mul(attn_sb[:, p], sm_e[:, p], sm_rec[:])

            # attn -> [co, p, s] (per-partition scalars for the aggregation)
            attn_T = mlpp.tile([COUT, P_PAT, BS_LOCAL], F32, tag="attn_T")
            for p in range(P_PAT):
                at_ps = ps_small.tile([COUT, BS_LOCAL], F32, tag="sm")
                nc.tensor.transpose(
                    at_ps[:], attn_sb[:, p], ident[0:BS_LOCAL, 0:BS_LOCAL])
                nc.vector.tensor_copy(attn_T[:, p], at_ps[:])

            # ---- phase A: aggregate + transpose for ALL samples ---------
            lhsTs = []
            for s in range(BS_LOCAL):
                agg = aggp.tile([128, CIN, KK], BF16, tag="agg")
                wmv = wm_sb[:].rearrange("co p (ci kl) -> co p ci kl", kl=KK)
                if "noagg" in K:
                    nc.vector.tensor_copy(
                        agg[:].rearrange("co ci kl -> co (ci kl)"), wm_sb[:, 0])
                else:
                    for k0, k1 in ((0, 5), (5, KK)):
                        nc.vector.tensor_scalar_mul(
                            agg[:, :, k0:k1], wmv[:, 0, :, k0:k1],
                            attn_T[:, 0, s:s + 1])
                        for p in range(1, P_PAT):
                            nc.vector.scalar_tensor_tensor(
                                agg[:, :, k0:k1],
                                wmv[:, p, :, k0:k1], attn_T[:, p, s:s + 1],
                                agg[:, :, k0:k1],
                                op0=mybir.AluOpType.mult,
                                op1=mybir.AluOpType.add)

                lhsT = lhsp.tile([CIN, KK, COUT], F32R, tag="lhsT",
                                 name=f"lhsT{s}")
                lhsTs.append(lhsT)
                if "notp" in K:
                    for kl in range(KK):
                        nc.scalar.copy(lhsT[:, kl], agg[:, :, kl])
                else:
                    for kl in range(KK):
                        tp_ps = ps_tp.tile([CIN, COUT], BF16, tag="tp_ps")
                        nc.tensor.transpose(tp_ps[:], agg[:, :, kl], identb[:])
                        nc.scalar.copy(lhsT[:, kl], tp_ps[:])

            ctx_front.close()

            # ---- phase B: convolutions ----------------------------------
            for s in range(BS_LOCAL):
                lhsT = lhsTs[s]
                pe_absorb(xs[s][:, 0, 0:1].bitcast(F32))
                n_tap = 1 if "noconv" in K else KK
                for yc in range(N_CHUNK):
                    y0 = yc * YC
                    pt = ps_mm.tile([COUT, YC, W], F32, tag="pt")
                    for i, (dk, dl) in enumerate(
                            (dk, dl) for dk in range(KS) for dl in range(KS)):
                        if i >= n_tap:
                            break
                        nc.tensor.matmul(
                            pt[:],
                            lhsT[:, dk * KS + dl],
                            xs[s][:, y0 + dk:y0 + dk + YC, dl:dl + W],
                            start=(i == 0), stop=(i == n_tap - 1))

                    ot = outp.tile([COUT, YC, W], BF16, tag="ot")
                    nc.scalar.copy(ot[:], pt[:])
                    nc.sync.dma_start(out_d[s, :, y0:y0 + YC, :], ot[:])

    _split_excess_waits(nc)
    return nc


def _split_excess_waits(nc, max_inline=1):
    """Hoist extra sync waits into standalone EventSemaphore instructions.

    This walrus build rejects instructions whose encoded sync-command
    count exceeds the ISA struct capacity ("Too many sync wait
    commands") -- in practice more than one wait per compute
    instruction. Engines execute their instruction stream in order, so
    blocking on a preceding same-engine EventSemaphore is equivalent to
    the instruction carrying the wait itself.
    """
    n = 0
    for f in nc.m.functions:
        for blk in f.blocks:
            out = []
            for inst in blk.instructions:
                si = inst.sync_info
                if si is not None and len(si.on_wait) > max_inline:
                    waits = list(si.on_wait)
                    keep = waits[:max_inline]
                    for w in waits[max_inline:]:
                        n += 1
                        ev = mybir.InstEventSemaphore(
                            name=f"WSPLIT-{n}", ins=[], outs=[])
                        ev.engine = inst.engine
                        ev.sync_info = mybir.SyncInfo(on_wait=[w], on_update=[])
                        ev.debug = inst.debug
                        nc.inst_map[ev.name] = ev
                        out.append(ev)
                    inst.sync_info = mybir.SyncInfo(
                        on_wait=keep, on_update=list(si.on_update))
                out.append(inst)
            blk.instructions = out
    return n


class _Runner:
    """Cached PJRT executor for the 8-core SPMD program.

    Mirrors bass2jax.run_bass_via_pjrt's multi-core path but keeps the
    jitted shard_map callable (and the device mesh) alive across calls,
    so repeat invocations skip retracing and recompilation.
    """

    def __init__(self, nc):
        import jax
        import jax.numpy as jnp
        from jax.experimental.shard_map import shard_map
        from jax.sharding import Mesh, PartitionSpec, NamedSharding
        from concourse import bass2jax, mybir as _mb

        bass2jax.install_neuronx_cc_hook()
        self.jax = jax
        self.nc = nc
        assert nc.dbg_addr is None

        partition_name = (nc.partition_id_tensor.name
                          if nc.partition_id_tensor else None)
        in_names, out_names, out_avals, zero_shapes = [], [], [], []
        for alloc in nc.m.functions[0].allocations:
            if not isinstance(alloc, _mb.MemoryLocationSet):
                continue
            name = alloc.memorylocations[0].name
            if alloc.kind == "ExternalInput":
                if name != partition_name:
                    in_names.append(name)
            elif alloc.kind == "ExternalOutput":
                out_names.append(name)
                shape = tuple(alloc.tensor_shape)
                dtype = _mb.dt.np(alloc.dtype)
                out_avals.append(jax.core.ShapedArray(shape, dtype))
                zero_shapes.append((shape, dtype))
        self.in_names = list(in_names)
        self.out_names = out_names
        self.out_avals = out_avals
        n_params = len(in_names)
        n_outs = len(out_names)
        all_in_names = in_names + out_names
        if partition_name is not None:
            all_in_names.append(partition_name)
        donate = tuple(range(n_params, n_params + n_outs))

        def _body(*args):
            operands = list(args)
            if partition_name is not None:
                operands.append(bass2jax.partition_id_tensor())
            outs = bass2jax._bass_exec_p.bind(
                *operands,
                out_avals=tuple(out_avals),
                in_names=tuple(all_in_names),
                out_names=tuple(out_names),
                lowering_input_output_aliases=(),
                sim_require_finite=False,
                sim_require_nnan=False,
                nc=nc,
            )
            return tuple(outs)

        devices = jax.devices()[:N_CORES]
        self.mesh = Mesh(np.asarray(devices), ("core",))
        self.sharding = NamedSharding(self.mesh, PartitionSpec("core"))
        in_specs = (PartitionSpec("core"),) * (n_params + n_outs)
        out_specs = (PartitionSpec("core"),) * n_outs
        self.sharded = jax.jit(
            shard_map(_body, mesh=self.mesh, in_specs=in_specs,
                      out_specs=out_specs, check_rep=False),
            donate_argnums=donate, keep_unused=True)
        self._zero_makers = [
            jax.jit(
                (lambda sh=sh, dt=dt: jnp.zeros((N_CORES * sh[0],) + sh[1:], dt)),
                out_shardings=self.sharding)
            for sh, dt in zero_shapes
        ]

    def put_inputs(self, in_maps):
        """Concat per-core inputs on axis 0 and upload sharded."""
        cat = [
            np.concatenate([np.asarray(m[name]) for m in in_maps], axis=0)
            for name in self.in_names
        ]
        return [self.jax.device_put(a, self.sharding) for a in cat]

    def run(self, dev_inputs):
        zeros = [zm() for zm in self._zero_makers]
        outs = self.sharded(*dev_inputs, *zeros)
        self.jax.block_until_ready(outs)
        return outs

    def results(self, outs):
        res = []
        for c in range(N_CORES):
            res.append({
                name: np.asarray(outs[i]).reshape(
                    N_CORES, *self.out_avals[i].shape)[c]
                for i, name in enumerate(self.out_names)
            })
        return res


_RUNNER_CACHE = {}


def _get_runner(use_f32r=True, rep=1, mm=None, knobs=()):
    key = (rep, tuple(sorted(knobs)))
    if key not in _RUNNER_CACHE:
        _RUNNER_CACHE[key] = _Runner(_get_nc(rep=rep, knobs=knobs))
    return _RUNNER_CACHE[key]


_NC_CACHE = {}


def _get_nc(use_f32r=True, rep=1, mm=None, knobs=()):
    key = (rep, tuple(sorted(knobs)))
    if key not in _NC_CACHE:
        _NC_CACHE[key] = build_nc(rep=rep, knobs=knobs)
    return _NC_CACHE[key]


def make_in_maps(x, psa_w1, psa_w2, psa_b2, weight, threshold):
    import ml_dtypes
    bf16 = ml_dtypes.bfloat16
    x = np.asarray(x, dtype=np.float32)
    xp = np.zeros((BS, CIN, H + 2, W + 2), np.float32)
    xp[:, :, 1:H + 1, 1:W + 1] = x
    if os.environ.get("DPP_XF32", "0") != "1":
        xp = xp.astype(bf16)

    w = np.asarray(weight, np.float32)
    thr = np.asarray(threshold, np.float32).reshape(P_PAT, COUT)
    # binary spatial mask from (weight, threshold) alone
    mw = np.abs(w).mean(axis=(1, 2))                  # [P, K, K]
    mt = thr.mean(axis=1)[:, None, None]              # [P, 1, 1]
    m = ((1.0 / (1.0 + np.exp(-(mw - mt))) - 0.5) > 0).astype(np.float32)
    wm = w * m[:, None, None]                         # [P, O, I, K, K]
    # aggregation layout: [co, p, ci, (k l)]
    wmr = np.ascontiguousarray(
        wm.transpose(1, 0, 2, 3, 4).reshape(COUT, P_PAT, CIN, KK)
    ).astype(bf16)

    w1t = np.ascontiguousarray(np.asarray(psa_w1, np.float32).T)
    w2b = np.ascontiguousarray(np.asarray(psa_w2, np.float32).T)
    b2v = np.ascontiguousarray(np.asarray(psa_b2, np.float32)[None, :])

    common = {
        "wm": wmr, "w1t": w1t, "w2b": w2b, "b2": b2v,
        "ident": np.eye(128, dtype=np.float32),
    }
    return [
        {"x": xp[c * BS_LOCAL:(c + 1) * BS_LOCAL], **common}
        for c in range(N_CORES)
    ]


def kernel(x, psa_w1, psa_w2, psa_b2, weight, threshold):
    in_maps = make_in_maps(x, psa_w1, psa_w2, psa_b2, weight, threshold)
    try:
        r = _get_runner()
        outs = r.run(r.put_inputs(in_maps))
        res = r.results(outs)
    except Exception:
        nc = _get_nc()
        res = run_bass_kernel_spmd(nc, in_maps, list(range(N_CORES))).results
    return np.concatenate(
        [res[c]["out"] for c in range(N_CORES)], axis=0).astype(np.float32)
